# revision 3
# baseline (speedup 1.0000x reference)
"""APPNP GNN on 8 TRN2 NeuronCores.

Node-partitioned graph parallel: 12500 dst nodes/core in a degree-grouped rank
layout; replicated node table u = (1-a)*dinv*z in HBM updated per step via
AllGather; per step dma_gather of u[src] for all in-edges (int16 idxs over 4
address windows), VectorE pairwise-tree segment reduction, then
z_new = dinv*zsum + (1-a)*dinv^2*z_old + a*h. Self-loops folded into the DVE
math. MLP encoder on TensorE (bf16, feature-major) + PE transpose; on-chip
log_softmax.
"""
import numpy as np

N = 100000
C = 8
PER_C = 12500
RANKS = 12672          # rank slots per core (99 cols of 128)
NCOLZ = 98             # z columns (12544 rank slots; 12500 real)
D = 64
HID = 256
IN = 512
K = 10
ALPHA = 0.1
WIN = 32704
TABLE_ROWS = C * RANKS  # 101376
NWIN = 4
WIN_SIZES = [WIN, WIN, WIN, TABLE_ROWS - 3 * WIN]
CALL_ROWS = 64
XCOLS = 12800

_cache = {}


def _plan(edge_index):
    src = edge_index[0].astype(np.int64)
    dst = edge_index[1].astype(np.int64)
    core_e = dst // PER_C

    deg_ref = np.bincount(dst, minlength=N).astype(np.float64) + 1.0
    dinv = (1.0 / np.sqrt(deg_ref)).astype(np.float32)

    # window counts with approximate (identity-rank) global ids, for grouping
    g_approx = RANKS * (src // PER_C) + (src % PER_C)
    w_approx = np.minimum(g_approx // WIN, NWIN - 1)
    cnt_approx = np.zeros((N, NWIN), np.int32)
    np.add.at(cnt_approx, (dst, w_approx), 1)

    node_of_rank = np.full((C, RANKS), -1, np.int64)
    rank_of_node = np.zeros(N, np.int64)
    for c in range(C):
        nodes = np.arange(c * PER_C, (c + 1) * PER_C)
        v = cnt_approx[nodes]
        order = np.argsort(-v[:, 0], kind="stable")
        for blk, w in ((2048, 1), (512, 2), (128, 3)):
            out = np.empty_like(order)
            pos = 0
            for b0 in range(0, PER_C, blk):
                idxs = order[b0:b0 + blk]
                o2 = idxs[np.argsort(-v[idxs, w], kind="stable")]
                out[pos:pos + len(o2)] = o2
                pos += len(o2)
            order = out
        node_of_rank[c, :PER_C] = nodes[order]
        rank_of_node[nodes[order]] = np.arange(PER_C)

    g_of_node = RANKS * (np.arange(N) // PER_C) + rank_of_node
    g_src = g_of_node[src]
    w_e = np.minimum(g_src // WIN, NWIN - 1)
    loc_e = (g_src - WIN * w_e).astype(np.int64)
    dr_e = rank_of_node[dst]

    cnt = np.zeros((C, 12544, NWIN), np.int32)
    np.add.at(cnt, (core_e, dr_e, w_e), 1)
    Tcw = cnt.reshape(C, NCOLZ, 128, NWIN).max(axis=2)
    T = Tcw.max(axis=0)
    T = np.maximum(((T + 1) // 2) * 2, 2).astype(np.int64)
    assert T.max() <= CALL_ROWS, T.max()

    # block order: window-major, T desc
    calls = []
    cur = None
    for w in range(NWIN):
        for j in np.argsort(-T[:, w], kind="stable"):
            t = int(T[j, w])
            if cur is None or cur["w"] != w or cur["rows"] + t > CALL_ROWS:
                cur = {"w": w, "rows": 0, "blocks": []}
                calls.append(cur)
            cur["blocks"].append((int(j), t, cur["rows"]))
            cur["rows"] += t
    total_rows = sum(cl["rows"] for cl in calls)

    row0_of_call = np.cumsum([0] + [cl["rows"] for cl in calls])[:-1]
    slot_base = np.zeros((NCOLZ, NWIN), np.int64)
    for ci, cl in enumerate(calls):
        for (j, t, lr) in cl["blocks"]:
            slot_base[j, cl["w"]] = row0_of_call[ci] + lr

    # pad (zero-row) local idx per window: rank 12544 of some core in window
    pad_local = []
    for w in range(NWIN):
        lo, hi = WIN * w, WIN * w + WIN_SIZES[w]
        found = -1
        for c in range(C):
            g = RANKS * c + 12544
            if lo <= g < hi:
                found = g - lo
                break
        assert found >= 0, w
        pad_local.append(found)

    gidx = np.zeros((C, 128, total_rows), np.int16)
    for ci, cl in enumerate(calls):
        r0 = row0_of_call[ci]
        gidx[:, :, r0:r0 + cl["rows"]] = pad_local[cl["w"]]

    # vectorized edge placement
    key = (core_e * 12544 + dr_e) * NWIN + w_e
    order = np.argsort(key, kind="stable")
    sk = key[order]
    grp_start = np.r_[0, np.flatnonzero(np.diff(sk)) + 1]
    grp_len = np.diff(np.r_[grp_start, len(sk)])
    cumcount = np.arange(len(sk)) - np.repeat(grp_start, grp_len)
    dr_s = dr_e[order]
    w_s = w_e[order]
    c_s = core_e[order]
    p_s = dr_s % 128
    j_s = dr_s // 128
    row_s = slot_base[j_s, w_s] + cumcount
    gidx[c_s, p_s, row_s] = loc_e[order].astype(np.int16)

    dinv_l = np.zeros((C, 128, NCOLZ), np.float32)
    for c in range(C):
        nr = node_of_rank[c, :12544]
        valid = nr >= 0
        dv = np.where(valid, dinv[np.maximum(nr, 0)], 0.0).astype(np.float32)
        dinv_l[c] = dv.reshape(NCOLZ, 128).T

    # wrap to dma_gather idx layout: slot i=(r*128+p) -> (i%16, i//16)
    # (p=g*16+q, r) -> row q, col r*8+g ; then replicate to 128 partitions
    R = total_rows
    t16 = gidx.transpose(0, 2, 1).reshape(C, R, 8, 16)
    plane16 = t16.transpose(0, 3, 1, 2).reshape(C, 16, R * 8)
    gidx_w = np.tile(plane16, (1, 8, 1))  # [C, 128, R*8]
    return dict(calls=calls, row0_of_call=row0_of_call, total_rows=total_rows,
                gidx=gidx_w, dinv_l=dinv_l, node_of_rank=node_of_rank)


def _build(calls, total_rows, stage="full"):
    from concourse import bacc, mybir, tile
    from concourse import library_config
    from concourse.masks import make_identity

    nc = bacc.Bacc(None, target_bir_lowering=False, debug=True,
                   num_swdge_queues=4)
    f32 = mybir.dt.float32
    bf16 = mybir.dt.bfloat16
    i16 = mybir.dt.int16
    ADD = mybir.AluOpType.add
    MUL = mybir.AluOpType.mult

    xt = nc.declare_dram_parameter("xt", [IN, XCOLS], f32, isOutput=False)
    w1 = nc.declare_dram_parameter("w1", [IN, HID], f32, isOutput=False)
    w2 = nc.declare_dram_parameter("w2", [HID, D], f32, isOutput=False)
    b1p = nc.declare_dram_parameter("b1p", [128, 2], f32, isOutput=False)
    b2p = nc.declare_dram_parameter("b2p", [D, 1], f32, isOutput=False)
    gidxp = nc.declare_dram_parameter("gidx", [128, total_rows * 8], i16, isOutput=False)
    dinvp = nc.declare_dram_parameter("dinvp", [128, NCOLZ], f32, isOutput=False)
    outp = nc.declare_dram_parameter("out", [12544, D], f32, isOutput=True)

    ubounce = nc.dram_tensor("ubounce", [RANKS, D], f32)
    tableAG = nc.dram_tensor("tableAG", [TABLE_ROWS, D], f32, addr_space="Shared")

    with tile.TileContext(nc) as tc:
        with (
            tc.tile_pool(name="persist", bufs=1) as pp,
            tc.tile_pool(name="msgs", bufs=3) as msgp,
            tc.tile_pool(name="mlp", bufs=2) as mp,
            tc.tile_pool(name="psum", bufs=2, space="PSUM") as psp,
        ):
            nc.gpsimd.load_library(library_config.mlp)

            dinv_sb = pp.tile([128, NCOLZ], f32)
            nc.sync.dma_start(out=dinv_sb[:], in_=dinvp[:, :])
            b1_sb = pp.tile([128, 2], f32)
            nc.sync.dma_start(out=b1_sb[:], in_=b1p[:, :])
            b2_sb = pp.tile([D, 1], f32)
            nc.sync.dma_start(out=b2_sb[:], in_=b2p[:, :])

            ss_sb = pp.tile([128, NCOLZ], f32)
            us_sb = pp.tile([128, NCOLZ], f32)
            nc.vector.tensor_tensor(out=ss_sb[:], in0=dinv_sb[:], in1=dinv_sb[:], op=MUL)
            nc.vector.tensor_scalar(us_sb[:], dinv_sb[:], (1.0 - ALPHA), None, MUL)
            nc.vector.tensor_scalar(ss_sb[:], ss_sb[:], (1.0 - ALPHA), None, MUL)

            ident = pp.tile([128, 128], f32)
            make_identity(nc, ident[:])

            w1_sb = pp.tile([128, 4 * HID], bf16)
            w2_sb = pp.tile([128, 2 * D], bf16)
            for kc in range(4):
                wtmp = mp.tile([128, HID], f32, tag="wtmp")
                nc.sync.dma_start(out=wtmp[:], in_=w1[kc * 128:(kc + 1) * 128, :])
                nc.vector.tensor_copy(out=w1_sb[:, kc * HID:(kc + 1) * HID], in_=wtmp[:])
            for kc in range(2):
                wtmp = mp.tile([128, HID], f32, tag="wtmp")
                nc.sync.dma_start(out=wtmp[:, 0:D], in_=w2[kc * 128:(kc + 1) * 128, :])
                nc.vector.tensor_copy(out=w2_sb[:, kc * D:(kc + 1) * D], in_=wtmp[:, 0:D])

            z_old = pp.tile([128, NCOLZ, D], f32, tag="zA")
            zsum = pp.tile([128, NCOLZ, D], f32, tag="zB")
            ah = pp.tile([128, NCOLZ, D], f32, tag="ah")
            u_t = pp.tile([128, RANKS // 128, D], f32, tag="ut")
            nc.vector.memset(u_t[:], 0.0)

            # ---- MLP ----
            CH = 512
            for ch in range(XCOLS // CH):
                col0 = ch * CH
                x_bf = mp.tile([128, 4, CH], bf16, tag="xbf")
                for kc in range(4):
                    x_f = mp.tile([128, CH], f32, tag="xf")
                    nc.sync.dma_start(out=x_f[:],
                                      in_=xt[kc * 128:(kc + 1) * 128, col0:col0 + CH])
                    nc.vector.tensor_copy(out=x_bf[:, kc:kc + 1, :].rearrange("p o c -> p (o c)"),
                                          in_=x_f[:])
                h1a = mp.tile([128, CH], bf16, tag="h1a")
                h1b = mp.tile([128, CH], bf16, tag="h1b")
                for half in range(2):
                    ps = psp.tile([128, CH], f32, tag="ps_h1")
                    for kc in range(4):
                        nc.tensor.matmul(
                            out=ps[:],
                            lhsT=w1_sb[:, kc * HID + half * 128: kc * HID + half * 128 + 128],
                            rhs=x_bf[:, kc:kc + 1, :].rearrange("p o c -> p (o c)"),
                            start=(kc == 0), stop=(kc == 3))
                    nc.scalar.activation(
                        out=(h1a if half == 0 else h1b)[:], in_=ps[:],
                        func=mybir.ActivationFunctionType.Relu,
                        bias=b1_sb[:, half:half + 1], scale=1.0)
                ps2 = psp.tile([D, CH], f32, tag="ps_z0")
                nc.tensor.matmul(out=ps2[:], lhsT=w2_sb[:, 0:D], rhs=h1a[:],
                                 start=True, stop=False)
                nc.tensor.matmul(out=ps2[:], lhsT=w2_sb[:, D:2 * D], rhs=h1b[:],
                                 start=False, stop=True)
                hfm = mp.tile([D, CH], f32, tag="hfm")
                nc.scalar.activation(out=hfm[:], in_=ps2[:],
                                     func=mybir.ActivationFunctionType.Identity,
                                     bias=b2_sb[:, 0:1], scale=1.0)
                for tt in range(CH // 128):
                    col = ch * (CH // 128) + tt
                    if col >= NCOLZ:
                        continue
                    pst = psp.tile([128, D], f32, tag="ps_t")
                    nc.tensor.transpose(out=pst[:],
                                        in_=hfm[:, tt * 128:(tt + 1) * 128],
                                        identity=ident[0:64, 0:64])
                    nc.vector.tensor_copy(
                        out=z_old[:, col:col + 1, :].rearrange("p o c -> p (o c)"),
                        in_=pst[:])

            nc.vector.tensor_scalar(ah[:], z_old[:], ALPHA, None, MUL)

            def bcast(ap2d):
                return ap2d.rearrange("p (c o) -> p c o", o=1).to_broadcast([128, NCOLZ, D])

            import os
            KK = K if stage == "full" else (0 if stage == "mlp" else 1)
            for step in range(KK):
                nc.vector.tensor_tensor(out=u_t[:, 0:NCOLZ, :], in0=z_old[:],
                                        in1=bcast(us_sb[:]), op=MUL)
                nc.sync.dma_start(
                    out=ubounce[:, :].rearrange("(a p) d -> p a d", p=128),
                    in_=u_t[:])
                nc.gpsimd.collective_compute(
                    "AllGather", mybir.AluOpType.bypass,
                    replica_groups=[list(range(C))],
                    ins=[ubounce.ap().opt()],
                    outs=[tableAG.ap().opt()],
                )

                first = set()
                for ci, cl in enumerate(calls):
                    w = cl["w"]
                    rows = cl["rows"]
                    r0 = int(sum(c2["rows"] for c2 in calls[:ci]))
                    msg = msgp.tile([128, CALL_ROWS, D], f32, tag="msg")
                    gtile = msgp.tile([128, CALL_ROWS * 8], i16, tag="gt")
                    nc.sync.dma_start(out=gtile[:, 0:rows * 8],
                                      in_=gidxp[:, r0 * 8:(r0 + rows) * 8])
                    nc.gpsimd.dma_gather(
                        out_ap=msg[:, 0:rows, :],
                        in_ap=tableAG[WIN * w: WIN * w + WIN_SIZES[w], :],
                        idxs_ap=gtile[:, 0:rows * 8],
                        num_idxs=rows * 128,
                        num_idxs_reg=rows * 128,
                        elem_size=D,
                        queue_num=ci % 4,
                        single_packet=False,
                    )
                    flat = msg[:].rearrange("p r d -> p (r d)")
                    for (j, t, lr) in cl["blocks"]:
                        base = lr * D
                        while t > 2:
                            h = t // 2
                            nc.vector.tensor_tensor(
                                out=flat[:, base:base + h * D],
                                in0=flat[:, base:base + h * D],
                                in1=flat[:, base + (t - h) * D:base + t * D],
                                op=ADD)
                            t = t - h
                        if j in first:
                            nc.vector.tensor_tensor(
                                out=flat[:, base:base + D],
                                in0=flat[:, base:base + D],
                                in1=flat[:, base + D:base + 2 * D], op=ADD)
                            nc.vector.tensor_tensor(
                                out=zsum[:, j:j + 1, :],
                                in0=zsum[:, j:j + 1, :],
                                in1=msg[:, lr:lr + 1, :], op=ADD)
                        else:
                            first.add(j)
                            nc.vector.tensor_tensor(
                                out=zsum[:, j:j + 1, :],
                                in0=msg[:, lr:lr + 1, :],
                                in1=msg[:, lr + 1:lr + 2, :], op=ADD)

                nc.vector.tensor_tensor(out=zsum[:], in0=zsum[:],
                                        in1=bcast(dinv_sb[:]), op=MUL)
                nc.vector.tensor_tensor(out=z_old[:], in0=z_old[:],
                                        in1=bcast(ss_sb[:]), op=MUL)
                nc.vector.tensor_tensor(out=zsum[:], in0=zsum[:], in1=z_old[:], op=ADD)
                nc.vector.tensor_tensor(out=zsum[:], in0=zsum[:], in1=ah[:], op=ADD)
                z_old, zsum = zsum, z_old

            if stage != "full":
                nc.sync.dma_start(
                    out=outp[:, :].rearrange("(a p) d -> p a d", p=128),
                    in_=z_old[:, 0:NCOLZ, :])
                skip_softmax = True
            else:
                skip_softmax = False
            # ---- log_softmax ----
            mx = pp.tile([128, NCOLZ], f32)
            if not skip_softmax:
              nc.vector.tensor_reduce(out=mx[:], in_=z_old[:],
                                    axis=mybir.AxisListType.X,
                                    op=mybir.AluOpType.max)
            if not skip_softmax:
              nc.vector.tensor_tensor(out=z_old[:], in0=z_old[:], in1=bcast(mx[:]),
                                    op=mybir.AluOpType.subtract)
              ex = zsum
              nc.scalar.activation(out=ex[:], in_=z_old[:],
                                 func=mybir.ActivationFunctionType.Exp)
              sm = pp.tile([128, NCOLZ], f32)
              nc.vector.tensor_reduce(out=sm[:], in_=ex[:],
                                    axis=mybir.AxisListType.X,
                                    op=mybir.AluOpType.add)
              lsm = pp.tile([128, NCOLZ], f32)
              nc.scalar.activation(out=lsm[:], in_=sm[:],
                                 func=mybir.ActivationFunctionType.Ln)
              nc.vector.tensor_tensor(out=z_old[:], in0=z_old[:], in1=bcast(lsm[:]),
                                    op=mybir.AluOpType.subtract)
              nc.sync.dma_start(
                out=outp[:, :].rearrange("(a p) d -> p a d", p=128),
                in_=z_old[:, 0:NCOLZ, :])
    nc.compile()
    return nc


def kernel(x, W1, b1, W2, b2, edge_index):
    x = np.asarray(x, np.float32)
    W1 = np.asarray(W1, np.float32)
    b1 = np.asarray(b1, np.float32)
    W2 = np.asarray(W2, np.float32)
    b2 = np.asarray(b2, np.float32)
    ei = np.asarray(edge_index)

    import os
    stage = os.environ.get("KSTAGE", "full")
    ckey = (ei.shape[1], int(ei[0, 0]), int(ei[1, -1]), int(ei[0].sum() % (1 << 31)), stage)
    if ckey not in _cache:
        plan = _plan(ei)
        nc = _build(plan["calls"], plan["total_rows"], stage)
        _cache[ckey] = (plan, nc)
    plan, nc = _cache[ckey]

    b1p = np.ascontiguousarray(b1.reshape(2, 128).T)
    b2p = np.ascontiguousarray(b2.reshape(D, 1))
    in_maps = []
    for c in range(C):
        nor = plan["node_of_rank"][c][:12544]
        valid = nor >= 0
        xs = np.zeros((XCOLS, IN), np.float32)
        xs[:12544][valid] = x[nor[valid]]
        in_maps.append({
            "xt": np.ascontiguousarray(xs.T),
            "w1": W1, "w2": W2, "b1p": b1p, "b2p": b2p,
            "gidx": plan["gidx"][c],
            "dinvp": np.ascontiguousarray(plan["dinv_l"][c]),
        })

    from concourse.bass_utils import run_bass_kernel_spmd
    res = run_bass_kernel_spmd(nc, in_maps, list(range(C))).results

    out = np.zeros((N, D), np.float32)
    for c in range(C):
        r = res[c]["out"]
        nor = plan["node_of_rank"][c][:12544]
        valid = nor >= 0
        out[nor[valid]] = r[valid]
    return out



# revision 5
# speedup vs baseline: 1.5730x; 1.5730x over previous
"""APPNP GNN on 8 TRN2 NeuronCores.

Node-partitioned graph parallel: 12500 dst nodes/core in a degree-grouped rank
layout; replicated node table u = (1-a)*dinv*z in HBM updated per step via
AllGather; per step dma_gather of u[src] for all in-edges (int16 idxs over 4
address windows), VectorE pairwise-tree segment reduction, then
z_new = dinv*zsum + (1-a)*dinv^2*z_old + a*h. Self-loops folded into the DVE
math. MLP encoder on TensorE (bf16, feature-major) + PE transpose; on-chip
log_softmax.
"""
import numpy as np

N = 100000
C = 8
PER_C = 12500
RANKS = 12672          # rank slots per core (99 cols of 128)
NCOLZ = 98             # z columns (12544 rank slots; 12500 real)
D = 64
HID = 256
IN = 512
K = 10
ALPHA = 0.1
WIN = 32704
TABLE_ROWS = C * RANKS  # 101376
NWIN = 4
WIN_SIZES = [WIN, WIN, WIN, TABLE_ROWS - 3 * WIN]
CALL_ROWS = 64
XCOLS = 12800

_cache = {}


def _plan(edge_index):
    src = edge_index[0].astype(np.int64)
    dst = edge_index[1].astype(np.int64)
    core_e = dst // PER_C

    deg_ref = np.bincount(dst, minlength=N).astype(np.float64) + 1.0
    dinv = (1.0 / np.sqrt(deg_ref)).astype(np.float32)

    # window counts with approximate (identity-rank) global ids, for grouping
    g_approx = RANKS * (src // PER_C) + (src % PER_C)
    w_approx = np.minimum(g_approx // WIN, NWIN - 1)
    cnt_approx = np.zeros((N, NWIN), np.int32)
    np.add.at(cnt_approx, (dst, w_approx), 1)

    node_of_rank = np.full((C, RANKS), -1, np.int64)
    rank_of_node = np.zeros(N, np.int64)
    for c in range(C):
        nodes = np.arange(c * PER_C, (c + 1) * PER_C)
        v = cnt_approx[nodes]
        order = np.argsort(-v[:, 0], kind="stable")
        for blk, w in ((2048, 1), (512, 2), (128, 3)):
            out = np.empty_like(order)
            pos = 0
            for b0 in range(0, PER_C, blk):
                idxs = order[b0:b0 + blk]
                o2 = idxs[np.argsort(-v[idxs, w], kind="stable")]
                out[pos:pos + len(o2)] = o2
                pos += len(o2)
            order = out
        node_of_rank[c, :PER_C] = nodes[order]
        rank_of_node[nodes[order]] = np.arange(PER_C)

    g_of_node = RANKS * (np.arange(N) // PER_C) + rank_of_node
    g_src = g_of_node[src]
    w_e = np.minimum(g_src // WIN, NWIN - 1)
    loc_e = (g_src - WIN * w_e).astype(np.int64)
    dr_e = rank_of_node[dst]

    cnt = np.zeros((C, 12544, NWIN), np.int32)
    np.add.at(cnt, (core_e, dr_e, w_e), 1)
    Tcw = cnt.reshape(C, NCOLZ, 128, NWIN).max(axis=2)
    T = Tcw.max(axis=0)
    T = np.maximum(((T + 1) // 2) * 2, 2).astype(np.int64)
    assert T.max() <= CALL_ROWS, T.max()

    # block order: window-major, T desc
    calls = []
    cur = None
    for w in range(NWIN):
        for j in np.argsort(-T[:, w], kind="stable"):
            t = int(T[j, w])
            if cur is None or cur["w"] != w or cur["rows"] + t > CALL_ROWS:
                cur = {"w": w, "rows": 0, "blocks": []}
                calls.append(cur)
            cur["blocks"].append((int(j), t, cur["rows"]))
            cur["rows"] += t
    total_rows = sum(cl["rows"] for cl in calls)

    row0_of_call = np.cumsum([0] + [cl["rows"] for cl in calls])[:-1]
    slot_base = np.zeros((NCOLZ, NWIN), np.int64)
    for ci, cl in enumerate(calls):
        for (j, t, lr) in cl["blocks"]:
            slot_base[j, cl["w"]] = row0_of_call[ci] + lr

    # pad (zero-row) local idx per window: rank 12544 of some core in window
    pad_local = []
    for w in range(NWIN):
        lo, hi = WIN * w, WIN * w + WIN_SIZES[w]
        found = -1
        for c in range(C):
            g = RANKS * c + 12544
            if lo <= g < hi:
                found = g - lo
                break
        assert found >= 0, w
        pad_local.append(found)

    gidx = np.zeros((C, 128, total_rows), np.int16)
    for ci, cl in enumerate(calls):
        r0 = row0_of_call[ci]
        gidx[:, :, r0:r0 + cl["rows"]] = pad_local[cl["w"]]

    # vectorized edge placement
    key = (core_e * 12544 + dr_e) * NWIN + w_e
    order = np.argsort(key, kind="stable")
    sk = key[order]
    grp_start = np.r_[0, np.flatnonzero(np.diff(sk)) + 1]
    grp_len = np.diff(np.r_[grp_start, len(sk)])
    cumcount = np.arange(len(sk)) - np.repeat(grp_start, grp_len)
    dr_s = dr_e[order]
    w_s = w_e[order]
    c_s = core_e[order]
    p_s = dr_s % 128
    j_s = dr_s // 128
    row_s = slot_base[j_s, w_s] + cumcount
    gidx[c_s, p_s, row_s] = loc_e[order].astype(np.int16)

    dinv_l = np.zeros((C, 128, NCOLZ), np.float32)
    for c in range(C):
        nr = node_of_rank[c, :12544]
        valid = nr >= 0
        dv = np.where(valid, dinv[np.maximum(nr, 0)], 0.0).astype(np.float32)
        dinv_l[c] = dv.reshape(NCOLZ, 128).T

    # wrap to dma_gather idx layout: slot i=(r*128+p) -> (i%16, i//16)
    # (p=g*16+q, r) -> row q, col r*8+g ; then replicate to 128 partitions
    R = total_rows
    t16 = gidx.transpose(0, 2, 1).reshape(C, R, 8, 16)
    plane16 = t16.transpose(0, 3, 1, 2).reshape(C, 16, R * 8)
    gidx_w = np.tile(plane16, (1, 8, 1))  # [C, 128, R*8]
    return dict(calls=calls, row0_of_call=row0_of_call, total_rows=total_rows,
                gidx=gidx_w, dinv_l=dinv_l, node_of_rank=node_of_rank)


def _build(calls, total_rows, stage="full"):
    from concourse import bacc, mybir, tile
    from concourse import library_config
    from concourse.masks import make_identity

    nc = bacc.Bacc(None, target_bir_lowering=False, debug=True,
                   num_swdge_queues=4)
    f32 = mybir.dt.float32
    bf16 = mybir.dt.bfloat16
    i16 = mybir.dt.int16
    ADD = mybir.AluOpType.add
    MUL = mybir.AluOpType.mult

    xt = nc.declare_dram_parameter("xt", [IN, XCOLS], f32, isOutput=False)
    w1 = nc.declare_dram_parameter("w1", [IN, HID], f32, isOutput=False)
    w2 = nc.declare_dram_parameter("w2", [HID, D], f32, isOutput=False)
    b1p = nc.declare_dram_parameter("b1p", [128, 2], f32, isOutput=False)
    b2p = nc.declare_dram_parameter("b2p", [D, 1], f32, isOutput=False)
    gidxp = nc.declare_dram_parameter("gidx", [128, total_rows * 8], i16, isOutput=False)
    dinvp = nc.declare_dram_parameter("dinvp", [128, NCOLZ], f32, isOutput=False)
    outp = nc.declare_dram_parameter("out", [12544, D], f32, isOutput=True)

    ubounce = nc.dram_tensor("ubounce", [RANKS, D], f32)
    tableAG = nc.dram_tensor("tableAG", [TABLE_ROWS, D], f32, addr_space="Shared")
    tableLw = [nc.dram_tensor(f"tableL{w}", [WIN_SIZES[w], D], f32)
               for w in range(NWIN)]

    with tile.TileContext(nc) as tc:
        with (
            tc.tile_pool(name="persist", bufs=1) as pp,
            tc.tile_pool(name="msgs", bufs=3) as msgp,
            tc.tile_pool(name="mlp", bufs=2) as mp,
            tc.tile_pool(name="psum", bufs=2, space="PSUM") as psp,
        ):
            nc.gpsimd.load_library(library_config.mlp)

            dinv_sb = pp.tile([128, NCOLZ], f32)
            nc.sync.dma_start(out=dinv_sb[:], in_=dinvp[:, :])
            b1_sb = pp.tile([128, 2], f32)
            nc.sync.dma_start(out=b1_sb[:], in_=b1p[:, :])
            b2_sb = pp.tile([D, 1], f32)
            nc.sync.dma_start(out=b2_sb[:], in_=b2p[:, :])

            ss_sb = pp.tile([128, NCOLZ], f32)
            us_sb = pp.tile([128, NCOLZ], f32)
            nc.vector.tensor_tensor(out=ss_sb[:], in0=dinv_sb[:], in1=dinv_sb[:], op=MUL)
            nc.vector.tensor_scalar(us_sb[:], dinv_sb[:], (1.0 - ALPHA), None, MUL)
            nc.vector.tensor_scalar(ss_sb[:], ss_sb[:], (1.0 - ALPHA), None, MUL)

            ident = pp.tile([128, 128], f32)
            make_identity(nc, ident[:])

            w1_sb = pp.tile([128, 4 * HID], bf16)
            w2_sb = pp.tile([128, 2 * D], bf16)
            for kc in range(4):
                wtmp = mp.tile([128, HID], f32, tag="wtmp")
                nc.sync.dma_start(out=wtmp[:], in_=w1[kc * 128:(kc + 1) * 128, :])
                nc.vector.tensor_copy(out=w1_sb[:, kc * HID:(kc + 1) * HID], in_=wtmp[:])
            for kc in range(2):
                wtmp = mp.tile([128, HID], f32, tag="wtmp")
                nc.sync.dma_start(out=wtmp[:, 0:D], in_=w2[kc * 128:(kc + 1) * 128, :])
                nc.vector.tensor_copy(out=w2_sb[:, kc * D:(kc + 1) * D], in_=wtmp[:, 0:D])

            z_old = pp.tile([128, NCOLZ, D], f32, tag="zA")
            zsum = pp.tile([128, NCOLZ, D], f32, tag="zB")
            ah = pp.tile([128, NCOLZ, D], f32, tag="ah")
            u_t = pp.tile([128, RANKS // 128, D], f32, tag="ut")
            nc.vector.memset(u_t[:], 0.0)

            # ---- MLP ----
            CH = 512
            for ch in range(XCOLS // CH):
                col0 = ch * CH
                x_bf = mp.tile([128, 4, CH], bf16, tag="xbf")
                for kc in range(4):
                    x_f = mp.tile([128, CH], f32, tag="xf")
                    nc.sync.dma_start(out=x_f[:],
                                      in_=xt[kc * 128:(kc + 1) * 128, col0:col0 + CH])
                    nc.vector.tensor_copy(out=x_bf[:, kc:kc + 1, :].rearrange("p o c -> p (o c)"),
                                          in_=x_f[:])
                h1a = mp.tile([128, CH], bf16, tag="h1a")
                h1b = mp.tile([128, CH], bf16, tag="h1b")
                for half in range(2):
                    ps = psp.tile([128, CH], f32, tag="ps_h1")
                    for kc in range(4):
                        nc.tensor.matmul(
                            out=ps[:],
                            lhsT=w1_sb[:, kc * HID + half * 128: kc * HID + half * 128 + 128],
                            rhs=x_bf[:, kc:kc + 1, :].rearrange("p o c -> p (o c)"),
                            start=(kc == 0), stop=(kc == 3))
                    nc.scalar.activation(
                        out=(h1a if half == 0 else h1b)[:], in_=ps[:],
                        func=mybir.ActivationFunctionType.Relu,
                        bias=b1_sb[:, half:half + 1], scale=1.0)
                ps2 = psp.tile([D, CH], f32, tag="ps_z0")
                nc.tensor.matmul(out=ps2[:], lhsT=w2_sb[:, 0:D], rhs=h1a[:],
                                 start=True, stop=False)
                nc.tensor.matmul(out=ps2[:], lhsT=w2_sb[:, D:2 * D], rhs=h1b[:],
                                 start=False, stop=True)
                hfm = mp.tile([D, CH], f32, tag="hfm")
                nc.scalar.activation(out=hfm[:], in_=ps2[:],
                                     func=mybir.ActivationFunctionType.Identity,
                                     bias=b2_sb[:, 0:1], scale=1.0)
                for tt in range(CH // 128):
                    col = ch * (CH // 128) + tt
                    if col >= NCOLZ:
                        continue
                    pst = psp.tile([128, D], f32, tag="ps_t")
                    nc.tensor.transpose(out=pst[:],
                                        in_=hfm[:, tt * 128:(tt + 1) * 128],
                                        identity=ident[0:64, 0:64])
                    nc.vector.tensor_copy(
                        out=z_old[:, col:col + 1, :].rearrange("p o c -> p (o c)"),
                        in_=pst[:])

            nc.vector.tensor_scalar(ah[:], z_old[:], ALPHA, None, MUL)

            def bcast(ap2d):
                return ap2d.rearrange("p (c o) -> p c o", o=1).to_broadcast([128, NCOLZ, D])

            import os
            KK = K if stage == "full" else (0 if stage == "mlp" else 1)
            for step in range(KK):
                nc.vector.tensor_tensor(out=u_t[:, 0:NCOLZ, :], in0=z_old[:],
                                        in1=bcast(us_sb[:]), op=MUL)
                nc.sync.dma_start(
                    out=ubounce[:, :].rearrange("(a p) d -> p a d", p=128),
                    in_=u_t[:])
                nc.gpsimd.collective_compute(
                    "AllGather", mybir.AluOpType.bypass,
                    replica_groups=[list(range(C))],
                    ins=[ubounce.ap().opt()],
                    outs=[tableAG.ap().opt()],
                )
                for cw in range(NWIN):
                    nc.sync.dma_start(
                        out=tableLw[cw][:, :],
                        in_=tableAG[WIN * cw: WIN * cw + WIN_SIZES[cw], :])

                first = set()
                for ci, cl in enumerate(calls):
                    w = cl["w"]
                    rows = cl["rows"]
                    r0 = int(sum(c2["rows"] for c2 in calls[:ci]))
                    msg = msgp.tile([128, CALL_ROWS, D], f32, tag="msg")
                    gtile = msgp.tile([128, CALL_ROWS * 8], i16, tag="gt")
                    nc.sync.dma_start(out=gtile[:, 0:rows * 8],
                                      in_=gidxp[:, r0 * 8:(r0 + rows) * 8])
                    nc.gpsimd.dma_gather(
                        out_ap=msg[:, 0:rows, :],
                        in_ap=tableLw[w][:, :],
                        idxs_ap=gtile[:, 0:rows * 8],
                        num_idxs=rows * 128,
                        num_idxs_reg=rows * 128,
                        elem_size=D,
                        queue_num=ci % 4,
                        single_packet=False,
                    )
                    flat = msg[:].rearrange("p r d -> p (r d)")
                    for (j, t, lr) in cl["blocks"]:
                        base = lr * D
                        while t > 2:
                            h = t // 2
                            nc.vector.tensor_tensor(
                                out=flat[:, base:base + h * D],
                                in0=flat[:, base:base + h * D],
                                in1=flat[:, base + (t - h) * D:base + t * D],
                                op=ADD)
                            t = t - h
                        if j in first:
                            nc.vector.tensor_tensor(
                                out=flat[:, base:base + D],
                                in0=flat[:, base:base + D],
                                in1=flat[:, base + D:base + 2 * D], op=ADD)
                            nc.vector.tensor_tensor(
                                out=zsum[:, j:j + 1, :],
                                in0=zsum[:, j:j + 1, :],
                                in1=msg[:, lr:lr + 1, :], op=ADD)
                        else:
                            first.add(j)
                            nc.vector.tensor_tensor(
                                out=zsum[:, j:j + 1, :],
                                in0=msg[:, lr:lr + 1, :],
                                in1=msg[:, lr + 1:lr + 2, :], op=ADD)

                nc.vector.tensor_tensor(out=zsum[:], in0=zsum[:],
                                        in1=bcast(dinv_sb[:]), op=MUL)
                nc.vector.tensor_tensor(out=z_old[:], in0=z_old[:],
                                        in1=bcast(ss_sb[:]), op=MUL)
                nc.vector.tensor_tensor(out=zsum[:], in0=zsum[:], in1=z_old[:], op=ADD)
                nc.vector.tensor_tensor(out=zsum[:], in0=zsum[:], in1=ah[:], op=ADD)
                z_old, zsum = zsum, z_old

            if stage != "full":
                nc.sync.dma_start(
                    out=outp[:, :].rearrange("(a p) d -> p a d", p=128),
                    in_=z_old[:, 0:NCOLZ, :])
                skip_softmax = True
            else:
                skip_softmax = False
            # ---- log_softmax ----
            mx = pp.tile([128, NCOLZ], f32)
            if not skip_softmax:
              nc.vector.tensor_reduce(out=mx[:], in_=z_old[:],
                                    axis=mybir.AxisListType.X,
                                    op=mybir.AluOpType.max)
            if not skip_softmax:
              nc.vector.tensor_tensor(out=z_old[:], in0=z_old[:], in1=bcast(mx[:]),
                                    op=mybir.AluOpType.subtract)
              ex = zsum
              nc.scalar.activation(out=ex[:], in_=z_old[:],
                                 func=mybir.ActivationFunctionType.Exp)
              sm = pp.tile([128, NCOLZ], f32)
              nc.vector.tensor_reduce(out=sm[:], in_=ex[:],
                                    axis=mybir.AxisListType.X,
                                    op=mybir.AluOpType.add)
              lsm = pp.tile([128, NCOLZ], f32)
              nc.scalar.activation(out=lsm[:], in_=sm[:],
                                 func=mybir.ActivationFunctionType.Ln)
              nc.vector.tensor_tensor(out=z_old[:], in0=z_old[:], in1=bcast(lsm[:]),
                                    op=mybir.AluOpType.subtract)
              nc.sync.dma_start(
                out=outp[:, :].rearrange("(a p) d -> p a d", p=128),
                in_=z_old[:, 0:NCOLZ, :])
    nc.compile()
    return nc


def kernel(x, W1, b1, W2, b2, edge_index):
    x = np.asarray(x, np.float32)
    W1 = np.asarray(W1, np.float32)
    b1 = np.asarray(b1, np.float32)
    W2 = np.asarray(W2, np.float32)
    b2 = np.asarray(b2, np.float32)
    ei = np.asarray(edge_index)

    import os
    stage = os.environ.get("KSTAGE", "full")
    ckey = (ei.shape[1], int(ei[0, 0]), int(ei[1, -1]), int(ei[0].sum() % (1 << 31)), stage)
    if ckey not in _cache:
        plan = _plan(ei)
        nc = _build(plan["calls"], plan["total_rows"], stage)
        _cache[ckey] = (plan, nc)
    plan, nc = _cache[ckey]

    b1p = np.ascontiguousarray(b1.reshape(2, 128).T)
    b2p = np.ascontiguousarray(b2.reshape(D, 1))
    in_maps = []
    for c in range(C):
        nor = plan["node_of_rank"][c][:12544]
        valid = nor >= 0
        xs = np.zeros((XCOLS, IN), np.float32)
        xs[:12544][valid] = x[nor[valid]]
        in_maps.append({
            "xt": np.ascontiguousarray(xs.T),
            "w1": W1, "w2": W2, "b1p": b1p, "b2p": b2p,
            "gidx": plan["gidx"][c],
            "dinvp": np.ascontiguousarray(plan["dinv_l"][c]),
        })

    from concourse.bass_utils import run_bass_kernel_spmd
    res = run_bass_kernel_spmd(nc, in_maps, list(range(C))).results

    out = np.zeros((N, D), np.float32)
    for c in range(C):
        r = res[c]["out"]
        nor = plan["node_of_rank"][c][:12544]
        valid = nor >= 0
        out[nor[valid]] = r[valid]
    return out



# revision 6
# speedup vs baseline: 1.6052x; 1.0204x over previous
"""APPNP GNN on 8 TRN2 NeuronCores.

Node-partitioned graph parallel: 12500 dst nodes/core in a degree-grouped rank
layout; replicated node table u = (1-a)*dinv*z in HBM updated per step via
AllGather; per step dma_gather of u[src] for all in-edges (int16 idxs over 4
address windows), VectorE pairwise-tree segment reduction, then
z_new = dinv*zsum + (1-a)*dinv^2*z_old + a*h. Self-loops folded into the DVE
math. MLP encoder on TensorE (bf16, feature-major) + PE transpose; on-chip
log_softmax.
"""
import numpy as np

N = 100000
C = 8
PER_C = 12500
RANKS = 12672          # rank slots per core (99 cols of 128)
NCOLZ = 98             # z columns (12544 rank slots; 12500 real)
D = 64
HID = 256
IN = 512
K = 10
ALPHA = 0.1
WIN = 32704
TABLE_ROWS = C * RANKS  # 101376
NWIN = 4
WIN_SIZES = [WIN, WIN, WIN, TABLE_ROWS - 3 * WIN]
CALL_ROWS = 64
XCOLS = 12800

_cache = {}


def _plan(edge_index):
    src = edge_index[0].astype(np.int64)
    dst = edge_index[1].astype(np.int64)
    core_e = dst // PER_C

    deg_ref = np.bincount(dst, minlength=N).astype(np.float64) + 1.0
    dinv = (1.0 / np.sqrt(deg_ref)).astype(np.float32)

    # window counts with approximate (identity-rank) global ids, for grouping
    g_approx = RANKS * (src // PER_C) + (src % PER_C)
    w_approx = np.minimum(g_approx // WIN, NWIN - 1)
    cnt_approx = np.zeros((N, NWIN), np.int32)
    np.add.at(cnt_approx, (dst, w_approx), 1)

    node_of_rank = np.full((C, RANKS), -1, np.int64)
    rank_of_node = np.zeros(N, np.int64)
    for c in range(C):
        nodes = np.arange(c * PER_C, (c + 1) * PER_C)
        v = cnt_approx[nodes]
        order = np.argsort(-v[:, 0], kind="stable")
        for blk, w in ((2048, 1), (512, 2), (128, 3)):
            out = np.empty_like(order)
            pos = 0
            for b0 in range(0, PER_C, blk):
                idxs = order[b0:b0 + blk]
                o2 = idxs[np.argsort(-v[idxs, w], kind="stable")]
                out[pos:pos + len(o2)] = o2
                pos += len(o2)
            order = out
        node_of_rank[c, :PER_C] = nodes[order]
        rank_of_node[nodes[order]] = np.arange(PER_C)

    g_of_node = RANKS * (np.arange(N) // PER_C) + rank_of_node
    g_src = g_of_node[src]
    w_e = np.minimum(g_src // WIN, NWIN - 1)
    loc_e = (g_src - WIN * w_e).astype(np.int64)
    dr_e = rank_of_node[dst]

    cnt = np.zeros((C, 12544, NWIN), np.int32)
    np.add.at(cnt, (core_e, dr_e, w_e), 1)
    Tcw = cnt.reshape(C, NCOLZ, 128, NWIN).max(axis=2)
    T = Tcw.max(axis=0)
    T = np.maximum(((T + 1) // 2) * 2, 2).astype(np.int64)
    assert T.max() <= CALL_ROWS, T.max()

    # block order: window-major, T desc
    calls = []
    cur = None
    for w in range(NWIN):
        for j in np.argsort(-T[:, w], kind="stable"):
            t = int(T[j, w])
            if cur is None or cur["w"] != w or cur["rows"] + t > CALL_ROWS:
                cur = {"w": w, "rows": 0, "blocks": []}
                calls.append(cur)
            cur["blocks"].append((int(j), t, cur["rows"]))
            cur["rows"] += t
    total_rows = sum(cl["rows"] for cl in calls)

    row0_of_call = np.cumsum([0] + [cl["rows"] for cl in calls])[:-1]
    slot_base = np.zeros((NCOLZ, NWIN), np.int64)
    for ci, cl in enumerate(calls):
        for (j, t, lr) in cl["blocks"]:
            slot_base[j, cl["w"]] = row0_of_call[ci] + lr

    # pad (zero-row) local idx per window: rank 12544 of some core in window
    pad_local = []
    for w in range(NWIN):
        lo, hi = WIN * w, WIN * w + WIN_SIZES[w]
        found = -1
        for c in range(C):
            g = RANKS * c + 12544
            if lo <= g < hi:
                found = g - lo
                break
        assert found >= 0, w
        pad_local.append(found)

    gidx = np.zeros((C, 128, total_rows), np.int16)
    for ci, cl in enumerate(calls):
        r0 = row0_of_call[ci]
        gidx[:, :, r0:r0 + cl["rows"]] = pad_local[cl["w"]]

    # vectorized edge placement
    key = (core_e * 12544 + dr_e) * NWIN + w_e
    order = np.argsort(key, kind="stable")
    sk = key[order]
    grp_start = np.r_[0, np.flatnonzero(np.diff(sk)) + 1]
    grp_len = np.diff(np.r_[grp_start, len(sk)])
    cumcount = np.arange(len(sk)) - np.repeat(grp_start, grp_len)
    dr_s = dr_e[order]
    w_s = w_e[order]
    c_s = core_e[order]
    p_s = dr_s % 128
    j_s = dr_s // 128
    row_s = slot_base[j_s, w_s] + cumcount
    gidx[c_s, p_s, row_s] = loc_e[order].astype(np.int16)

    dinv_l = np.zeros((C, 128, NCOLZ), np.float32)
    for c in range(C):
        nr = node_of_rank[c, :12544]
        valid = nr >= 0
        dv = np.where(valid, dinv[np.maximum(nr, 0)], 0.0).astype(np.float32)
        dinv_l[c] = dv.reshape(NCOLZ, 128).T

    # wrap to dma_gather idx layout: slot i=(r*128+p) -> (i%16, i//16)
    # (p=g*16+q, r) -> row q, col r*8+g ; then replicate to 128 partitions
    R = total_rows
    t16 = gidx.transpose(0, 2, 1).reshape(C, R, 8, 16)
    plane16 = t16.transpose(0, 3, 1, 2).reshape(C, 16, R * 8)
    gidx_w = np.tile(plane16, (1, 8, 1))  # [C, 128, R*8]
    return dict(calls=calls, row0_of_call=row0_of_call, total_rows=total_rows,
                gidx=gidx_w, dinv_l=dinv_l, node_of_rank=node_of_rank)


def _build(calls, total_rows, stage="full"):
    from concourse import bacc, mybir, tile
    from concourse import library_config
    from concourse.masks import make_identity

    nc = bacc.Bacc(None, target_bir_lowering=False, debug=True,
                   num_swdge_queues=4)
    f32 = mybir.dt.float32
    bf16 = mybir.dt.bfloat16
    i16 = mybir.dt.int16
    ADD = mybir.AluOpType.add
    MUL = mybir.AluOpType.mult

    xt = nc.declare_dram_parameter("xt", [IN, XCOLS], f32, isOutput=False)
    w1 = nc.declare_dram_parameter("w1", [IN, HID], f32, isOutput=False)
    w2 = nc.declare_dram_parameter("w2", [HID, D], f32, isOutput=False)
    b1p = nc.declare_dram_parameter("b1p", [128, 2], f32, isOutput=False)
    b2p = nc.declare_dram_parameter("b2p", [D, 1], f32, isOutput=False)
    gidxp = nc.declare_dram_parameter("gidx", [128, total_rows * 8], i16, isOutput=False)
    dinvp = nc.declare_dram_parameter("dinvp", [128, NCOLZ], f32, isOutput=False)
    outp = nc.declare_dram_parameter("out", [12544, D], f32, isOutput=True)

    ubounce = nc.dram_tensor("ubounce", [RANKS, D], f32)
    tableAG = nc.dram_tensor("tableAG", [TABLE_ROWS, D], f32, addr_space="Shared")
    tableLw = [nc.dram_tensor(f"tableL{w}", [WIN_SIZES[w], D], f32)
               for w in range(NWIN)]

    with tile.TileContext(nc) as tc:
        with (
            tc.tile_pool(name="persist", bufs=1) as pp,
            tc.tile_pool(name="msgs", bufs=4) as msgp,
            tc.tile_pool(name="mlp", bufs=2) as mp,
            tc.tile_pool(name="psum", bufs=2, space="PSUM") as psp,
        ):
            nc.gpsimd.load_library(library_config.mlp)

            dinv_sb = pp.tile([128, NCOLZ], f32)
            nc.sync.dma_start(out=dinv_sb[:], in_=dinvp[:, :])
            b1_sb = pp.tile([128, 2], f32)
            nc.sync.dma_start(out=b1_sb[:], in_=b1p[:, :])
            b2_sb = pp.tile([D, 1], f32)
            nc.sync.dma_start(out=b2_sb[:], in_=b2p[:, :])

            ss_sb = pp.tile([128, NCOLZ], f32)
            us_sb = pp.tile([128, NCOLZ], f32)
            nc.vector.tensor_tensor(out=ss_sb[:], in0=dinv_sb[:], in1=dinv_sb[:], op=MUL)
            nc.vector.tensor_scalar(us_sb[:], dinv_sb[:], (1.0 - ALPHA), None, MUL)
            nc.vector.tensor_scalar(ss_sb[:], ss_sb[:], (1.0 - ALPHA), None, MUL)

            ident = pp.tile([128, 128], f32)
            make_identity(nc, ident[:])

            w1_sb = pp.tile([128, 4 * HID], bf16)
            w2_sb = pp.tile([128, 2 * D], bf16)
            for kc in range(4):
                wtmp = mp.tile([128, HID], f32, tag="wtmp")
                nc.sync.dma_start(out=wtmp[:], in_=w1[kc * 128:(kc + 1) * 128, :])
                nc.vector.tensor_copy(out=w1_sb[:, kc * HID:(kc + 1) * HID], in_=wtmp[:])
            for kc in range(2):
                wtmp = mp.tile([128, HID], f32, tag="wtmp")
                nc.sync.dma_start(out=wtmp[:, 0:D], in_=w2[kc * 128:(kc + 1) * 128, :])
                nc.vector.tensor_copy(out=w2_sb[:, kc * D:(kc + 1) * D], in_=wtmp[:, 0:D])

            z_old = pp.tile([128, NCOLZ, D], f32, tag="zA")
            zsum = pp.tile([128, NCOLZ, D], f32, tag="zB")
            ah = pp.tile([128, NCOLZ, D], f32, tag="ah")
            u_t = pp.tile([128, RANKS // 128, D], f32, tag="ut")
            nc.vector.memset(u_t[:], 0.0)

            # ---- MLP ----
            CH = 512
            for ch in range(XCOLS // CH):
                col0 = ch * CH
                x_bf = mp.tile([128, 4, CH], bf16, tag="xbf")
                for kc in range(4):
                    x_f = mp.tile([128, CH], f32, tag="xf")
                    nc.sync.dma_start(out=x_f[:],
                                      in_=xt[kc * 128:(kc + 1) * 128, col0:col0 + CH])
                    nc.vector.tensor_copy(out=x_bf[:, kc:kc + 1, :].rearrange("p o c -> p (o c)"),
                                          in_=x_f[:])
                h1a = mp.tile([128, CH], bf16, tag="h1a")
                h1b = mp.tile([128, CH], bf16, tag="h1b")
                for half in range(2):
                    ps = psp.tile([128, CH], f32, tag="ps_h1")
                    for kc in range(4):
                        nc.tensor.matmul(
                            out=ps[:],
                            lhsT=w1_sb[:, kc * HID + half * 128: kc * HID + half * 128 + 128],
                            rhs=x_bf[:, kc:kc + 1, :].rearrange("p o c -> p (o c)"),
                            start=(kc == 0), stop=(kc == 3))
                    nc.scalar.activation(
                        out=(h1a if half == 0 else h1b)[:], in_=ps[:],
                        func=mybir.ActivationFunctionType.Relu,
                        bias=b1_sb[:, half:half + 1], scale=1.0)
                ps2 = psp.tile([D, CH], f32, tag="ps_z0")
                nc.tensor.matmul(out=ps2[:], lhsT=w2_sb[:, 0:D], rhs=h1a[:],
                                 start=True, stop=False)
                nc.tensor.matmul(out=ps2[:], lhsT=w2_sb[:, D:2 * D], rhs=h1b[:],
                                 start=False, stop=True)
                hfm = mp.tile([D, CH], f32, tag="hfm")
                nc.scalar.activation(out=hfm[:], in_=ps2[:],
                                     func=mybir.ActivationFunctionType.Identity,
                                     bias=b2_sb[:, 0:1], scale=1.0)
                for tt in range(CH // 128):
                    col = ch * (CH // 128) + tt
                    if col >= NCOLZ:
                        continue
                    pst = psp.tile([128, D], f32, tag="ps_t")
                    nc.tensor.transpose(out=pst[:],
                                        in_=hfm[:, tt * 128:(tt + 1) * 128],
                                        identity=ident[0:64, 0:64])
                    nc.vector.tensor_copy(
                        out=z_old[:, col:col + 1, :].rearrange("p o c -> p (o c)"),
                        in_=pst[:])

            nc.vector.tensor_scalar(ah[:], z_old[:], ALPHA, None, MUL)

            def bcast(ap2d):
                return ap2d.rearrange("p (c o) -> p c o", o=1).to_broadcast([128, NCOLZ, D])

            import os
            KK = K if stage == "full" else (0 if stage == "mlp" else 1)
            for step in range(KK):
                nc.vector.tensor_tensor(out=u_t[:, 0:NCOLZ, :], in0=z_old[:],
                                        in1=bcast(us_sb[:]), op=MUL)
                nc.sync.dma_start(
                    out=ubounce[:, :].rearrange("(a p) d -> p a d", p=128),
                    in_=u_t[:])
                nc.gpsimd.collective_compute(
                    "AllGather", mybir.AluOpType.bypass,
                    replica_groups=[list(range(C))],
                    ins=[ubounce.ap().opt()],
                    outs=[tableAG.ap().opt()],
                )
                for cw in range(NWIN):
                    nc.sync.dma_start(
                        out=tableLw[cw][:, :],
                        in_=tableAG[WIN * cw: WIN * cw + WIN_SIZES[cw], :])

                first = set()
                for ci, cl in enumerate(calls):
                    w = cl["w"]
                    rows = cl["rows"]
                    r0 = int(sum(c2["rows"] for c2 in calls[:ci]))
                    msg = msgp.tile([128, CALL_ROWS, D], f32, tag="msg")
                    gtile = msgp.tile([128, CALL_ROWS * 8], i16, tag="gt")
                    nc.sync.dma_start(out=gtile[:, 0:rows * 8],
                                      in_=gidxp[:, r0 * 8:(r0 + rows) * 8])
                    nc.gpsimd.dma_gather(
                        out_ap=msg[:, 0:rows, :],
                        in_ap=tableLw[w][:, :],
                        idxs_ap=gtile[:, 0:rows * 8],
                        num_idxs=rows * 128,
                        num_idxs_reg=rows * 128,
                        elem_size=D,
                        queue_num=ci % 4,
                        single_packet=False,
                    )
                    flat = msg[:].rearrange("p r d -> p (r d)")
                    for (j, t, lr) in cl["blocks"]:
                        base = lr * D
                        while t > 2:
                            h = t // 2
                            nc.vector.tensor_tensor(
                                out=flat[:, base:base + h * D],
                                in0=flat[:, base:base + h * D],
                                in1=flat[:, base + (t - h) * D:base + t * D],
                                op=ADD)
                            t = t - h
                        if j in first:
                            nc.vector.tensor_tensor(
                                out=flat[:, base:base + D],
                                in0=flat[:, base:base + D],
                                in1=flat[:, base + D:base + 2 * D], op=ADD)
                            nc.vector.tensor_tensor(
                                out=zsum[:, j:j + 1, :],
                                in0=zsum[:, j:j + 1, :],
                                in1=msg[:, lr:lr + 1, :], op=ADD)
                        else:
                            first.add(j)
                            nc.vector.tensor_tensor(
                                out=zsum[:, j:j + 1, :],
                                in0=msg[:, lr:lr + 1, :],
                                in1=msg[:, lr + 1:lr + 2, :], op=ADD)

                nc.vector.tensor_tensor(out=zsum[:], in0=zsum[:],
                                        in1=bcast(dinv_sb[:]), op=MUL)
                nc.vector.tensor_tensor(out=z_old[:], in0=z_old[:],
                                        in1=bcast(ss_sb[:]), op=MUL)
                nc.vector.tensor_tensor(out=zsum[:], in0=zsum[:], in1=z_old[:], op=ADD)
                nc.vector.tensor_tensor(out=zsum[:], in0=zsum[:], in1=ah[:], op=ADD)
                z_old, zsum = zsum, z_old

            if stage != "full":
                nc.sync.dma_start(
                    out=outp[:, :].rearrange("(a p) d -> p a d", p=128),
                    in_=z_old[:, 0:NCOLZ, :])
                skip_softmax = True
            else:
                skip_softmax = False
            # ---- log_softmax ----
            mx = pp.tile([128, NCOLZ], f32)
            if not skip_softmax:
              nc.vector.tensor_reduce(out=mx[:], in_=z_old[:],
                                    axis=mybir.AxisListType.X,
                                    op=mybir.AluOpType.max)
            if not skip_softmax:
              nc.vector.tensor_tensor(out=z_old[:], in0=z_old[:], in1=bcast(mx[:]),
                                    op=mybir.AluOpType.subtract)
              ex = zsum
              nc.scalar.activation(out=ex[:], in_=z_old[:],
                                 func=mybir.ActivationFunctionType.Exp)
              sm = pp.tile([128, NCOLZ], f32)
              nc.vector.tensor_reduce(out=sm[:], in_=ex[:],
                                    axis=mybir.AxisListType.X,
                                    op=mybir.AluOpType.add)
              lsm = pp.tile([128, NCOLZ], f32)
              nc.scalar.activation(out=lsm[:], in_=sm[:],
                                 func=mybir.ActivationFunctionType.Ln)
              nc.vector.tensor_tensor(out=z_old[:], in0=z_old[:], in1=bcast(lsm[:]),
                                    op=mybir.AluOpType.subtract)
              nc.sync.dma_start(
                out=outp[:, :].rearrange("(a p) d -> p a d", p=128),
                in_=z_old[:, 0:NCOLZ, :])
    nc.compile()
    return nc


def kernel(x, W1, b1, W2, b2, edge_index):
    x = np.asarray(x, np.float32)
    W1 = np.asarray(W1, np.float32)
    b1 = np.asarray(b1, np.float32)
    W2 = np.asarray(W2, np.float32)
    b2 = np.asarray(b2, np.float32)
    ei = np.asarray(edge_index)

    import os
    stage = os.environ.get("KSTAGE", "full")
    ckey = (ei.shape[1], int(ei[0, 0]), int(ei[1, -1]), int(ei[0].sum() % (1 << 31)), stage)
    if ckey not in _cache:
        plan = _plan(ei)
        nc = _build(plan["calls"], plan["total_rows"], stage)
        _cache[ckey] = (plan, nc)
    plan, nc = _cache[ckey]

    b1p = np.ascontiguousarray(b1.reshape(2, 128).T)
    b2p = np.ascontiguousarray(b2.reshape(D, 1))
    in_maps = []
    for c in range(C):
        nor = plan["node_of_rank"][c][:12544]
        valid = nor >= 0
        xs = np.zeros((XCOLS, IN), np.float32)
        xs[:12544][valid] = x[nor[valid]]
        in_maps.append({
            "xt": np.ascontiguousarray(xs.T),
            "w1": W1, "w2": W2, "b1p": b1p, "b2p": b2p,
            "gidx": plan["gidx"][c],
            "dinvp": np.ascontiguousarray(plan["dinv_l"][c]),
        })

    from concourse.bass_utils import run_bass_kernel_spmd
    res = run_bass_kernel_spmd(nc, in_maps, list(range(C))).results

    out = np.zeros((N, D), np.float32)
    for c in range(C):
        r = res[c]["out"]
        nor = plan["node_of_rank"][c][:12544]
        valid = nor >= 0
        out[nor[valid]] = r[valid]
    return out

